# revision 17
# baseline (speedup 1.0000x reference)
"""Trainium2 Bass kernel for nn_BQuantConv1d.

Math (reference):
    sign[k,f,8g+j] = 2*bit_{7-j}(binary[k,f,g]) - 1
    W[f,n]  = sum_k scale[k,f] * sign[k,f,n]          (NF=4096, NX=1024)
    out     = x @ W.T + bias                          (x: (2,2048,1024))

Sharding: NF split across 8 cores (512 features each); x replicated.

Host marshaling (layout/cast only): xT[n, t] = bf16(x[t, n]) in
bit-position chunk order so the GEMM stationary operand loads straight
from SBUF (kills 256 PE transposes and halves the 16MB x read); binary
pre-packed to u8; scalet/dmat are the scale transpose / diag expansion.

Per-core plan (engine assignment tuned so the decode head is short):
  DMA queues: sync = byts per f-tile then xT token batches then half the
    out stores; scalar = scalet/ident/bias; gpsimd = dmat per f-tile.
  Decode: DVE extracts bits as u16 pairs of fp8e4 {0, 2.0} lanes into a
    k-major per-f-tile layout (ft0/ft1 on DVE, ft2/ft3 on GpSimd so the
    DVE queue doesn't gate the later f-tiles); one N=512 matmul per
    (ft, k, j-half): psum[f, (j,g)] += D_k.T @ bits.  Scalar ACT
    evacuates W = psum - C[f] (per-partition bias) in bf16.
  W transposes: chunk c holds bit position j=c (BT[g, c, f] =
    W[f, 8g+c]); the GEMM contracts chunk c against the matching xT
    chunk, so the permuted order cancels.
  GEMM: per 128-token tile: 8 N=512 matmuls out_ps = sum_c xT_c.T @
    BT_c with lhsT read directly from the xT SBUF region.  DVE adds
    broadcast bias on PSUM evacuation; out stores alternate queues.
"""

import sys

sys.path.insert(0, "/opt/trn_rl_repo")

import numpy as np
import ml_dtypes
import concourse.bass as bass
import concourse.mybir as mybir
import concourse.tile as tile
from concourse import bacc
from concourse.bass_utils import run_bass_kernel_spmd

F32 = mybir.dt.float32
BF16 = mybir.dt.bfloat16
I32 = mybir.dt.int32
U8 = mybir.dt.uint8
U16 = mybir.dt.uint16
FP8 = mybir.dt.float8e4
Alu = mybir.AluOpType
Ax = mybir.AxisListType

NCORES = 8
T = 4096  # tokens (2*2048)
NX = 1024
KB = 8  # bit planes
G = 128  # packed groups per row (NX/8)
NFL = 512  # features per core (4096/8)
NFT = NFL // 128  # f-tiles per core = 4
TT = T // 128  # token tiles = 32
NC = NX // 128  # contraction chunks = 8

_CACHED = {}


def _build_nc():
    nc = bacc.Bacc(None, target_bir_lowering=False, debug=False)

    # host-transposed bf16 x in bit-position chunk order
    xt_d = nc.dram_tensor("xt", [NX, T], BF16, kind="ExternalInput")
    # host-packed sign bytes: byts[p, ft, k, g] = binary[k, ft*128+p, g]
    byts_d = nc.dram_tensor("byts", [128, NFT, KB, G], U8, kind="ExternalInput")
    # host-pre-transposed scale: scalet[p, ft*KB+k] = scale[k, ft*128+p]
    scalet_d = nc.dram_tensor("scalet", [128, NFT * KB], F32, kind="ExternalInput")
    # host-built constants: bf16 identity, D[p,(ft,k,c)] = diag(scale) bf16
    ident_d = nc.dram_tensor("ident", [128, 128], BF16, kind="ExternalInput")
    dmat_d = nc.dram_tensor("dmat", [128, NFT * KB * 128], BF16, kind="ExternalInput")
    bias_d = nc.dram_tensor("bias", [1, NFL], F32, kind="ExternalInput")
    out_d = nc.dram_tensor("out", [T, NFL], F32, kind="ExternalOutput")

    with tile.TileContext(nc) as tc:
        with (
            tc.tile_pool(name="const", bufs=1) as cpool,
            tc.tile_pool(name="x_sb", bufs=10) as xpool,
            tc.tile_pool(name="out_sb", bufs=3) as opool,
            tc.tile_pool(name="bits", bufs=4) as bpool,
            tc.tile_pool(name="wf_sb", bufs=2) as wfpool,
            tc.tile_pool(name="dec_ps", bufs=2, space="PSUM") as dps,
            tc.tile_pool(name="tr_ps", bufs=2, space="PSUM") as xtps,
            tc.tile_pool(name="out_ps", bufs=3, space="PSUM") as ops,
            tc.tile_pool(name="warm_ps", bufs=1, space="PSUM") as wps,
        ):
            # ---- PE warm-up: ~3.4us of dummy matmuls while DMAs land so
            # the HAM clock gate is at 8/8 when the first decode matmul
            # issues (cold N=512 matmuls run at 1.2GHz, ~1.6us penalty)
            warm_sb = cpool.tile([128, 128], BF16, name="warm")
            nc.vector.memset(warm_sb, 1.0)
            warm_psum = wps.tile([128, 128], F32, name="warmps")
            for _ in range(40):
                nc.tensor.matmul(
                    warm_psum, warm_sb, warm_sb, start=True, stop=True
                )
            # ---- tiny dummy DMAs absorb each queue's first-DMA ring-setup
            # latency (~1-2us) so the byts0 transfer that gates the first
            # extract isn't paying it
            for eng in (nc.sync, nc.scalar, nc.gpsimd):
                dwarm = cpool.tile([1, 16], U8, name=f"dmaw_{eng.engine}")
                eng.dma_start(dwarm, byts_d[0:1, 0, 0, 0:16])

            # ---- packed sign bytes first on sync (decode gate); one tile
            # per f-tile so extracts depend only on their own DMA
            byts_tiles = []
            for ft in range(NFT):
                bt_ = cpool.tile([128, KB * G], U8, name=f"byts{ft}")
                nc.sync.dma_start(
                    bt_.rearrange("p (k g) -> p k g", k=KB), byts_d[:, ft]
                )
                byts_tiles.append(
                    bt_.rearrange("p (k w) -> p k w", k=KB).bitcast(U16)
                )

            # ---- D0 + small inputs on the scalar queue (issues in
            # parallel with byts; D0 gates the very first decode matmul,
            # and the gpsimd SWDGE queue is ~3us slower to first byte)
            dmat_v = dmat_d.rearrange("p (t k c) -> p t k c", t=NFT, k=KB)
            D_tiles = []
            for ft in range(NFT):
                dt_ = cpool.tile([128, KB * 128], BF16, name=f"D{ft}")
                D_tiles.append(dt_.rearrange("p (k c) -> p k c", k=KB))
            nc.scalar.dma_start(D_tiles[0], dmat_v[:, 0])
            scalet = cpool.tile([128, NFT * KB], F32)
            nc.scalar.dma_start(scalet, scalet_d[:, :])
            ident_bf = cpool.tile([128, 128], BF16)
            nc.scalar.dma_start(ident_bf, ident_d[:, :])
            bias_f = cpool.tile([1, NFL], F32)
            nc.scalar.dma_start(bias_f, bias_d[:, :])

            # ---- remaining diag(scale) tiles on the gpsimd queue
            for ft in range(1, NFT):
                nc.gpsimd.dma_start(D_tiles[ft], dmat_v[:, ft])

            # ---- xT prefetch in token batches behind byts on sync
            XBATCH = [1, 1, 2, 4, 4, 4, 4, 4, 4, 4]
            assert sum(XBATCH) == TT
            xt_view = xt_d.rearrange("(c p) t -> p c t", p=128)
            xt_of_tt = []
            t0 = 0
            for bi, xb in enumerate(XBATCH):
                x_bf = xpool.tile([128, NC, xb * 128], BF16, name=f"xt{bi}", tag="x_bf")
                nc.sync.dma_start(
                    x_bf, xt_view[:, :, t0 * 128 : (t0 + xb) * 128]
                )
                for a in range(xb):
                    xt_of_tt.append(x_bf[:, :, a * 128 : (a + 1) * 128])
                t0 += xb

            # ---- bit extraction: u16 pairs of fp8e4 {0, 2.0=0x40}; one
            # tensor_scalar per (ft, j) in 2x mode, k-major layout
            # bits_ft[f, k*KB*G + j*G + g].  All on DVE (Pool rejects
            # bitwise tensor_scalar); negC slots after ex0 so the ft0
            # psum evacuation isn't gated behind all four extract groups.
            bits_tiles = {}

            def extract_ft(ft, eng):
                bb = bpool.tile([128, KB * KB * G // 2], U16, name=f"bb{ft}", tag="bits")
                bb_v = bb.rearrange("p (k j w) -> p k j w", k=KB, j=KB)
                src = byts_tiles[ft]  # [p, k, w]
                for j in range(KB):
                    s = 7 - j
                    msk = (1 << s) | (1 << (s + 8))
                    if s == 7:
                        eng.tensor_scalar(
                            bb_v[:, :, j], src, msk, 1,
                            op0=Alu.bitwise_and, op1=Alu.logical_shift_right,
                        )
                    else:
                        eng.tensor_scalar(
                            bb_v[:, :, j], src, msk, 6 - s,
                            op0=Alu.bitwise_and, op1=Alu.logical_shift_left,
                        )
                bits_tiles[ft] = bb.bitcast(FP8).rearrange(
                    "p (k c) -> p k c", k=KB
                )

            extract_ft(0, nc.vector)

            # negC[f, ft] = -sum_k scale (exact f32), on DVE ahead of ex1
            negC = cpool.tile([128, NFT], F32)
            for ft in range(NFT):
                nc.vector.tensor_reduce(
                    negC[:, ft : ft + 1],
                    scalet[:, ft * KB : (ft + 1) * KB],
                    axis=Ax.X,
                    op=Alu.add,
                    negate=True,
                )

            extract_ft(1, nc.vector)
            extract_ft(2, nc.vector)
            extract_ft(3, nc.vector)

            # ---- W.T in bit-position chunks: BT[g, c, f] = W[f, 8g+c]
            BT = cpool.tile([128, NC, NFL], BF16)

            # decode one (f-tile, j-half): one N=512 matmul per k streams
            # bits_ft[:, k, jh-half]; psum[f, (j,g)] = 2*sum_k s_k*b.
            wf_tiles = {}

            def decode_half(ft, jh):
                bb = bits_tiles[ft]
                hs = slice(jh * 4 * G, (jh + 1) * 4 * G)
                psum_w = dps.tile([128, 4 * G], F32, name=f"wps{ft}_{jh}", tag="wps")
                for k in range(KB):
                    nc.tensor.matmul(
                        psum_w,
                        D_tiles[ft][:, k],
                        bb[:, k, hs],
                        start=(k == 0),
                        stop=(k == KB - 1),
                    )
                if jh == 0:
                    wf_tiles[ft] = wfpool.tile(
                        [128, KB * G], BF16, name=f"wf{ft}", tag="wf"
                    )
                # W[f, (j,g)] = psum - C[f]; scalar ACT with per-partition
                # bias, bf16 out (keeps the DVE queue clear)
                nc.scalar.add(wf_tiles[ft][:, hs], psum_w, negC[:, ft : ft + 1])

            # transpose one f-tile of W into BT chunks (chunk c = bit j=c);
            # DVE evacuates in chunk-halves so the last f-tile's copy
            # overlaps its second-half transposes (gates the first GEMM mm)
            def transpose_wf(ft):
                wf = wf_tiles.pop(ft)
                ps = xtps.tile([128, NC * 128], BF16, name=f"btp{ft}", tag="tr_ps")
                ps_v = ps.rearrange("p (c f) -> p c f", c=NC)
                for c in range(NC):
                    nc.tensor.transpose(
                        ps[:, c * 128 : (c + 1) * 128],
                        wf[:, c * G : (c + 1) * G],
                        ident_bf,
                    )
                    if c == NC // 2 - 1:
                        nc.vector.tensor_copy(
                            BT[:, : NC // 2, ft * 128 : (ft + 1) * 128],
                            ps_v[:, : NC // 2],
                        )
                nc.vector.tensor_copy(
                    BT[:, NC // 2 :, ft * 128 : (ft + 1) * 128],
                    ps_v[:, NC // 2 :],
                )

            # ---- decode schedule: j-halves back-to-back (psum double-
            # buffered), W transposes trailing one f-tile
            for ft in range(NFT):
                decode_half(ft, 0)
                decode_half(ft, 1)
                if ft > 0:
                    transpose_wf(ft - 1)
            transpose_wf(NFT - 1)

            # ---- bias broadcast tile [128, NFL] via rank-1 ones matmul
            ones_row = cpool.tile([1, 128], BF16)
            nc.vector.memset(ones_row, 1.0)
            bias_bf = cpool.tile([1, NFL], BF16)
            nc.vector.tensor_copy(bias_bf, bias_f)
            bias_bc = cpool.tile([128, NFL], F32)
            ps_b = ops.tile([128, NFL], F32, tag="out_ps")
            nc.tensor.matmul(ps_b, ones_row, bias_bf, start=True, stop=True)
            nc.vector.tensor_copy(bias_bc, ps_b)

            out_pair = {}

            def gemm_block(tt):
                xt_sb = xt_of_tt[tt]
                out_ps = ops.tile([128, NFL], F32, name=f"op{tt}", tag="out_ps")
                for c in range(NC):
                    nc.tensor.matmul(
                        out_ps,
                        xt_sb[:, c, :],
                        BT[:, c, :],
                        start=(c == 0),
                        stop=(c == NC - 1),
                    )
                # stage 2 token tiles per store (halves the out-DMA count)
                # except the last two, stored singly to shorten the tail.
                # All stores on the scalar queue: sync is busy issuing xT.
                single = tt >= TT - 2
                if tt % 2 == 0 or single:
                    out_pair[0] = opool.tile(
                        [128, 1 if single else 2, NFL],
                        F32, name=f"os{tt}", tag="out_sb",
                    )
                out_sb = out_pair[0]
                nc.vector.tensor_add(
                    out_sb[:, 0 if single else tt % 2, :], out_ps, bias_bc
                )
                if tt == TT - 1:
                    # final tile split across both queues in parallel:
                    # halves the store latency on the critical tail
                    dst = out_d[tt * 128 : (tt + 1) * 128, :].rearrange(
                        "(a p) f -> p a f", a=1
                    )
                    nc.scalar.dma_start(dst[:64], out_sb[:64])
                    nc.sync.dma_start(dst[64:], out_sb[64:])
                elif tt % 2 == 1 or single:
                    a = 1 if single else 2
                    dst = out_d[(tt - a + 1) * 128 : (tt + 1) * 128, :].rearrange(
                        "(a p) f -> p a f", a=a
                    )
                    nc.scalar.dma_start(dst, out_sb)

            for tt in range(TT):
                gemm_block(tt)

    nc.finalize()
    return nc


def _to_bf16(a):
    return np.asarray(a, dtype=np.float32).astype(ml_dtypes.bfloat16)


def _install_ntff_hook():
    """The agent image's antenv lacks axon_hooks; synthesize it so
    run_bass_kernel_spmd(trace=True) can capture NTFF profiles."""
    import types

    if "antenv.axon_hooks" in sys.modules:
        return
    import antenv
    from trn_agent_boot.trn_boot import _ntff_profile_via_ctypes

    mod = types.ModuleType("antenv.axon_hooks")
    state = {"hook": _ntff_profile_via_ctypes("/opt/axon/libaxon_pjrt.so")}
    mod.set_axon_ntff_profile_hook = lambda h: state.__setitem__("hook", h)
    mod.get_axon_ntff_profile_hook = lambda: state["hook"]
    sys.modules["antenv.axon_hooks"] = mod
    antenv.axon_hooks = mod


def kernel(x, binary, scale, bias, _trace=False):
    x = np.ascontiguousarray(np.asarray(x), dtype=np.float32)
    binary = np.ascontiguousarray(np.asarray(binary), dtype=np.int32)
    scale = np.ascontiguousarray(np.asarray(scale), dtype=np.float32)
    bias = np.ascontiguousarray(np.asarray(bias), dtype=np.float32)

    orig_shape = x.shape[:-1] + (binary.shape[1],)
    xf = x.reshape(-1, x.shape[-1])
    if scale.ndim == 3:
        scale = scale[..., 0]  # (KB, NF)

    if "nc" not in _CACHED:
        _CACHED["nc"] = _build_nc()
    nc = _CACHED["nc"]

    ident_np = np.eye(128, dtype=ml_dtypes.bfloat16)
    # xT in bit-position chunk order to match BT: row c*128+g = x[:, 8g+c]
    xtT = np.ascontiguousarray(
        _to_bf16(xf).T.reshape(G, KB, T).transpose(1, 0, 2).reshape(NX, T)
    )

    in_maps = []
    for i in range(NCORES):
        fsl = slice(i * NFL, (i + 1) * NFL)
        # byts[p, ft, k, g] = binary[k, ft*128+p, g] & 0xFF
        bslice = binary[:, fsl, :].astype(np.uint8)  # (KB, NFL, G)
        byts = np.ascontiguousarray(
            bslice.reshape(KB, NFT, 128, G).transpose(2, 1, 0, 3)
        )
        # scalet[p, ft*KB+k] = scale[k, ft*128+p]
        sc = scale[:, fsl].reshape(KB, NFT, 128)
        scalet = np.ascontiguousarray(
            sc.transpose(2, 1, 0).reshape(128, NFT * KB)
        )
        # dmat[p, (ft,k,c)] = diag over (p,c) of scale[k, ft*128+p], bf16
        dmat = _to_bf16(scalet)[:, :, None] * ident_np[:, None, :]
        dmat = np.ascontiguousarray(dmat.reshape(128, NFT * KB * 128))
        in_maps.append(
            {
                "xt": xtT,
                "byts": byts,
                "scalet": scalet,
                "ident": ident_np,
                "dmat": dmat,
                "bias": bias[fsl].reshape(1, NFL),
            }
        )

    kw = {}
    if _trace:
        _install_ntff_hook()
        kw = dict(trace=True, trace_cores=[0])
    res = run_bass_kernel_spmd(nc, in_maps, core_ids=list(range(NCORES)), **kw)
    out = np.concatenate([res.results[i]["out"] for i in range(NCORES)], axis=1)
    if _trace:
        return out.reshape(orig_shape), res
    return out.reshape(orig_shape)
